# revision 1
# baseline (speedup 1.0000x reference)
# Trainium2 Bass kernel for the MindForge LoRA head problem.
#
# Computation (see reference):
#   h0      = context @ ctx_w.T + ctx_b          (B, H)
#   h       = gelu(LN(h0) * ln_g + ln_b)         (B, H)
#   coeffs  = h @ coeff_w.T + coeff_b            (B, 8)
#   y       = x @ A_flat.T                       (B, 32)   A_flat[(n,r),d] = basis_A[n,r,d]
#   z_br    = sum_n coeffs_bn * y_b(n,r)         (B, 4)
#   u       = coeffs_bn * z_br                   (B, 32)
#   out     = x @ base_w.T + base_b + u @ Bmat   (B, C)    Bmat[(n,r),c] = basis_B[n,c,r]
#
# Distribution: column-parallel over num_classes. Each of the 8 cores computes
# a CS=6400-wide padded shard of the output columns; the tiny ctx-MLP / LoRA
# coefficient pipeline is replicated on every core. base_b is folded into the
# LoRA matmul as a 33rd row of [u | 1] @ [Bmat ; base_b].
#
# All matmuls run as float32r (TF32, full PE rate). The big matmul keeps x^T
# resident in SBUF (per 1024-row batch half) and streams base_w^T through a
# double-buffered panel, accumulating 16 K-tiles + the K=33 LoRA/bias tile
# into one PSUM bank per output tile.

import numpy as np
from contextlib import ExitStack

import concourse.bass as bass
import concourse.tile as tile
from concourse import bacc, mybir
from concourse.bass_utils import run_bass_kernel_spmd
from concourse.masks import make_identity

F32 = mybir.dt.float32
F32R = mybir.dt.float32r
AF = mybir.ActivationFunctionType
AX = mybir.AxisListType

D = 2048          # d_model
B = 2048          # batch
C_FULL = 50257    # num_classes
NB = 8            # n_basis
RK = 4            # rank
H = 128           # hidden
N_CORES = 8
CS = 6400         # per-core padded class shard (8*6400 = 51200 >= 50257)
LN_EPS = 1e-5

KT = D // 128           # 16 k-tiles
B_HALF = B // 2         # 1024
BT_HALF = B_HALF // 128  # 8 b-tiles per half
NT = 512                # main free-dim tile width
C_TILES = [512] * 12 + [256]   # sums to 6400
assert sum(C_TILES) == CS


def _emit_half(nc, P, g, u, hf, main_only=False):
    """Emit one batch-half: load x^T, coefficient pipeline, main matmul.

    P: dict of pools, g: dict of global/const tiles + dram APs,
    u: unique name prefix (per rep), hf: which half.
    """
    b0 = hf * B_HALF
    d_xT, d_ctxT, d_bwT, d_Bm, d_out = (g["d_xT"], g["d_ctxT"], g["d_bwT"],
                                        g["d_Bm"], g["d_out"])
    ident, cwT, aT, coefw, ctxb, lng, lnb, coefb = (
        g["ident"], g["cwT"], g["aT"], g["coefw"], g["ctxb"], g["lng"],
        g["lnb"], g["coefb"])

    # =========== phase A: coefficient pipeline (replicated) ===========
    if main_only:
        xt = P["xpool"].tile([128, KT * B_HALF], F32R, tag="xt", name=f"xt_{u}")
        for k in range(KT):
            nc.sync.dma_start(xt[:, k * B_HALF:(k + 1) * B_HALF],
                              d_xT[k * 128:(k + 1) * 128, b0:b0 + B_HALF])
        c0 = 0
        for ci, W in enumerate(C_TILES):
            panel = P["bwp"].tile([128, KT * NT], F32R, tag="bw", name=f"bw_{u}_{ci}")
            for k in range(KT):
                nc.sync.dma_start(panel[:, k * NT:k * NT + W],
                                  d_bwT[k * 128:(k + 1) * 128, c0:c0 + W])
            for b in range(BT_HALF):
                po = P["psM"].tile([128, NT], F32, tag="out", name=f"po_{u}_{ci}_{b}")
                for k in range(KT):
                    nc.tensor.matmul(
                        po[:, :W],
                        xt[:, k * B_HALF + b * 128:k * B_HALF + (b + 1) * 128],
                        panel[:, k * NT:k * NT + W],
                        start=(k == 0), stop=(k == KT - 1))
                ot = P["outp"].tile([128, NT], F32, tag="ot", name=f"ot_{u}_{ci}_{b}")
                nc.vector.tensor_copy(ot[:, :W], po[:, :W])
                nc.sync.dma_start(
                    d_out[b0 + b * 128:b0 + (b + 1) * 128, c0:c0 + W],
                    ot[:, :W])
            c0 += W
        return
    h0 = P["sbA"].tile([128, B_HALF], F32, tag="h0", name=f"h0_{u}")
    hT = P["sbA"].tile([128, B_HALF], F32R, tag="hT", name=f"hT_{u}")
    cfs = P["sbA"].tile([128, BT_HALF * NB], F32, tag="cfs", name=f"cfs_{u}")
    ysb = P["sbA"].tile([128, BT_HALF * 32], F32, tag="ysb", name=f"ysb_{u}")
    uT33 = P["sbA"].tile([NB * RK + 1, B_HALF], F32R, tag="uT33",
                         name=f"uT33_{u}")

    # A1: h0^T = ctx_w @ context^T  (H on partitions), then transpose
    for bc in range(B_HALF // NT):
        acc = P["psA"].tile([128, NT], F32, tag="acc", name=f"h0T_ps_{u}_{bc}")
        for k in range(KT):
            cx = P["cstr"].tile([128, NT], F32R, tag="cx", name=f"cx_{u}_{bc}_{k}")
            nc.sync.dma_start(cx[:], d_ctxT[k * 128:(k + 1) * 128,
                                            b0 + bc * NT:b0 + (bc + 1) * NT])
            nc.tensor.matmul(acc[:], cwT[:, k * H:(k + 1) * H], cx[:],
                             start=(k == 0), stop=(k == KT - 1))
        h0T = P["sbA"].tile([128, NT], F32, tag="h0T", name=f"h0T_{u}_{bc}")
        # psum -> sbuf adding ctx_b (per-partition: h is the partition dim)
        nc.scalar.activation(h0T[:], acc[:], AF.Identity, bias=ctxb[:])
        for j in range(NT // 128):
            t = bc * (NT // 128) + j
            tr = P["psA"].tile([128, 128], F32, tag="tr", name=f"h0tr_{u}_{t}")
            nc.tensor.transpose(tr[:], h0T[:, j * 128:(j + 1) * 128], ident[:])
            nc.vector.tensor_copy(h0[:, t * 128:(t + 1) * 128], tr[:])

    # ---- resident x^T for this half: k-major [128, KT*1024] ----
    xt = P["xpool"].tile([128, KT * B_HALF], F32R, tag="xt", name=f"xt_{u}")
    for k in range(KT):
        nc.sync.dma_start(xt[:, k * B_HALF:(k + 1) * B_HALF],
                          d_xT[k * 128:(k + 1) * 128, b0:b0 + B_HALF])

    # A4: y^T = A_flat @ x^T, then transpose to y (batch on partitions)
    for bc in range(B_HALF // NT):
        acc = P["psA"].tile([32, NT], F32, tag="acc", name=f"yT_ps_{u}_{bc}")
        for k in range(KT):
            nc.tensor.matmul(acc[:], aT[:, k * 32:(k + 1) * 32],
                             xt[:, k * B_HALF + bc * NT:k * B_HALF + (bc + 1) * NT],
                             start=(k == 0), stop=(k == KT - 1))
        yT = P["sbA"].tile([32, NT], F32, tag="yT", name=f"yT_{u}_{bc}")
        nc.vector.tensor_copy(yT[:], acc[:])
        for j in range(NT // 128):
            t = bc * (NT // 128) + j
            tr = P["psA"].tile([128, 32], F32, tag="tr", name=f"ytr_{u}_{t}")
            nc.tensor.transpose(tr[:], yT[:, j * 128:(j + 1) * 128],
                                ident[:32, :32])
            nc.vector.tensor_copy(ysb[:, t * 32:(t + 1) * 32], tr[:])

    # A2: LayerNorm + gelu per b-tile (batch on partitions)
    for t in range(BT_HALF):
        blk = h0[:, t * 128:(t + 1) * 128]
        mu = P["small"].tile([128, 1], F32, tag="mu", name=f"mu_{u}_{t}")
        s2 = P["small"].tile([128, 1], F32, tag="s2", name=f"s2_{u}_{t}")
        sq = P["sbA"].tile([128, H], F32, tag="sq", name=f"sq_{u}_{t}")
        nc.vector.reduce_sum(mu[:], blk, axis=AX.X)
        nc.scalar.activation(sq[:], blk, AF.Square, accum_out=s2[:])
        nc.vector.tensor_scalar_mul(mu[:], mu[:], 1.0 / H)
        nc.vector.tensor_scalar_mul(s2[:], s2[:], 1.0 / H)
        mu2 = P["small"].tile([128, 1], F32, tag="mu2", name=f"mu2_{u}_{t}")
        nc.vector.tensor_mul(mu2[:], mu[:], mu[:])
        var = P["small"].tile([128, 1], F32, tag="var", name=f"var_{u}_{t}")
        nc.vector.tensor_sub(var[:], s2[:], mu2[:])
        nc.vector.tensor_scalar_add(var[:], var[:], LN_EPS)
        std = P["small"].tile([128, 1], F32, tag="std", name=f"std_{u}_{t}")
        nc.scalar.sqrt(std[:], var[:])
        rstd = P["small"].tile([128, 1], F32, tag="rstd", name=f"rstd_{u}_{t}")
        nc.vector.reciprocal(rstd[:], std[:])
        nmr = P["small"].tile([128, 1], F32, tag="nmr", name=f"nmr_{u}_{t}")
        nc.vector.tensor_mul(nmr[:], mu[:], rstd[:])
        nc.vector.tensor_scalar_mul(nmr[:], nmr[:], -1.0)
        hn = P["sbA"].tile([128, H], F32, tag="hn", name=f"hn_{u}_{t}")
        nc.scalar.activation(hn[:], blk, AF.Identity, bias=nmr[:], scale=rstd[:])
        nc.vector.tensor_mul(hn[:], hn[:], lng[:])
        nc.vector.tensor_add(hn[:], hn[:], lnb[:])
        # gelu (erf variant) back into h0 block
        nc.scalar.activation(blk, hn[:], AF.Gelu)

    # A3: transpose h -> hT, coeffs = h @ coeff_w^T + coeff_b
    for t in range(BT_HALF):
        tr = P["psA"].tile([128, 128], F32, tag="tr", name=f"htr_{u}_{t}")
        nc.tensor.transpose(tr[:], h0[:, t * 128:(t + 1) * 128], ident[:])
        nc.vector.tensor_copy(hT[:, t * 128:(t + 1) * 128], tr[:])
        cf = P["psA"].tile([128, NB], F32, tag="tr", name=f"cf_ps_{u}_{t}")
        nc.tensor.matmul(cf[:], hT[:, t * 128:(t + 1) * 128], coefw[:],
                         start=True, stop=True)
        nc.vector.tensor_add(cfs[:, t * NB:(t + 1) * NB], cf[:], coefb[:])

    # A5: z = sum_n coeffs*y ; u = coeffs (x) z ; uT33 = [u | 1]^T
    for t in range(BT_HALF):
        yb = ysb[:, t * 32:(t + 1) * 32]
        cb = cfs[:, t * NB:(t + 1) * NB]
        prod = P["sbA"].tile([128, 32], F32, tag="prod", name=f"prod_{u}_{t}")
        # prod stored r-major: prod[p, r*8+n] = y[p, n*4+r] * coeffs[p, n]
        nc.vector.tensor_mul(
            prod[:].rearrange("p (r n) -> p r n", n=NB),
            yb.rearrange("p (n r) -> p r n", r=RK),
            cb.unsqueeze(1).broadcast_to((128, RK, NB)))
        z = P["small"].tile([128, RK], F32, tag="z", name=f"z_{u}_{t}")
        nc.vector.reduce_sum(z[:], prod[:].rearrange("p (r n) -> p r n", n=NB),
                             axis=AX.X)
        ut = P["sbA"].tile([128, NB * RK + 1], F32, tag="u", name=f"u_{u}_{t}")
        nc.vector.tensor_mul(
            ut[:, :NB * RK].rearrange("p (n r) -> p n r", r=RK),
            cb.unsqueeze(2).broadcast_to((128, NB, RK)),
            z[:].unsqueeze(1).broadcast_to((128, NB, RK)))
        nc.gpsimd.memset(ut[:, NB * RK:NB * RK + 1], 1.0)
        tr = P["psA"].tile([NB * RK + 1, 128], F32, tag="tr", name=f"utr_{u}_{t}")
        nc.tensor.transpose(tr[:], ut[:], ident[:])
        nc.vector.tensor_copy(uT33[:, t * 128:(t + 1) * 128], tr[:])

    # =========== phase B: main column-parallel matmul ===========
    c0 = 0
    for ci, W in enumerate(C_TILES):
        panel = P["bwp"].tile([128, KT * NT], F32R, tag="bw", name=f"bw_{u}_{ci}")
        for k in range(KT):
            nc.sync.dma_start(panel[:, k * NT:k * NT + W],
                              d_bwT[k * 128:(k + 1) * 128, c0:c0 + W])
        bx = P["bxp"].tile([NB * RK + 1, NT], F32R, tag="bx", name=f"bx_{u}_{ci}")
        nc.sync.dma_start(bx[:, :W], d_Bm[:, c0:c0 + W])
        for b in range(BT_HALF):
            po = P["psM"].tile([128, NT], F32, tag="out", name=f"po_{u}_{ci}_{b}")
            for k in range(KT):
                nc.tensor.matmul(
                    po[:, :W],
                    xt[:, k * B_HALF + b * 128:k * B_HALF + (b + 1) * 128],
                    panel[:, k * NT:k * NT + W],
                    start=(k == 0), stop=False)
            nc.tensor.matmul(po[:, :W], uT33[:, b * 128:(b + 1) * 128],
                             bx[:, :W], start=False, stop=True)
            ot = P["outp"].tile([128, NT], F32, tag="ot", name=f"ot_{u}_{ci}_{b}")
            nc.vector.tensor_copy(ot[:, :W], po[:, :W])
            nc.sync.dma_start(
                d_out[b0 + b * 128:b0 + (b + 1) * 128, c0:c0 + W],
                ot[:, :W])
        c0 += W


def _build_program(reps=1, main_only=False):
    nc = bacc.Bacc("TRN2", target_bir_lowering=False, debug=False,
                   num_devices=N_CORES)

    g = {}
    # DRAM I/O (per-core shapes; inputs marked f32r feed the PE directly)
    g["d_xT"] = nc.dram_tensor("xT", [D, B], F32R, kind="ExternalInput").ap()
    g["d_ctxT"] = nc.dram_tensor("ctxT", [D, B], F32R, kind="ExternalInput").ap()
    d_ctx_wT = nc.dram_tensor("ctx_wT", [D, H], F32R, kind="ExternalInput").ap()
    d_ctx_b = nc.dram_tensor("ctx_b_col", [H, 1], F32, kind="ExternalInput").ap()
    d_ln_g = nc.dram_tensor("ln_g_bc", [128, H], F32, kind="ExternalInput").ap()
    d_ln_b = nc.dram_tensor("ln_b_bc", [128, H], F32, kind="ExternalInput").ap()
    d_cw = nc.dram_tensor("coeff_wT", [H, NB], F32R, kind="ExternalInput").ap()
    d_cb = nc.dram_tensor("coeff_b_bc", [128, NB], F32, kind="ExternalInput").ap()
    d_AT = nc.dram_tensor("A_flatT", [D, NB * RK], F32R, kind="ExternalInput").ap()
    g["d_bwT"] = nc.dram_tensor("bwT", [D, CS], F32R, kind="ExternalInput").ap()
    g["d_Bm"] = nc.dram_tensor("Bm33", [NB * RK + 1, CS], F32R,
                               kind="ExternalInput").ap()
    g["d_out"] = nc.dram_tensor("out", [B, CS], F32, kind="ExternalOutput").ap()

    with tile.TileContext(nc) as tc, ExitStack() as ctx:
        P = {}
        P["const"] = ctx.enter_context(tc.tile_pool(name="const", bufs=1))
        P["sbA"] = ctx.enter_context(tc.tile_pool(name="sbA", bufs=2))
        P["small"] = ctx.enter_context(tc.tile_pool(name="small", bufs=4))
        P["cstr"] = ctx.enter_context(tc.tile_pool(name="cstr", bufs=4))
        P["xpool"] = ctx.enter_context(tc.tile_pool(name="xpool", bufs=1))
        P["bwp"] = ctx.enter_context(tc.tile_pool(name="bwp", bufs=2))
        P["bxp"] = ctx.enter_context(tc.tile_pool(name="bxp", bufs=2))
        P["outp"] = ctx.enter_context(tc.tile_pool(name="outp", bufs=6))
        P["psA"] = ctx.enter_context(tc.tile_pool(name="psA", bufs=2, space="PSUM"))
        P["psM"] = ctx.enter_context(tc.tile_pool(name="psM", bufs=4, space="PSUM"))

        # ---- constants / replicated small tensors ----
        ident = P["const"].tile([128, 128], F32, name="ident")
        make_identity(nc, ident[:])
        g["ident"] = ident
        cwT = P["const"].tile([128, KT * H], F32R, name="cwT")   # ctx_w^T k-major
        for k in range(KT):
            nc.sync.dma_start(cwT[:, k * H:(k + 1) * H],
                              d_ctx_wT[k * 128:(k + 1) * 128, :])
        g["cwT"] = cwT
        aT = P["const"].tile([128, KT * NB * RK], F32R, name="aT")  # A_flat^T
        for k in range(KT):
            nc.sync.dma_start(aT[:, k * 32:(k + 1) * 32],
                              d_AT[k * 128:(k + 1) * 128, :])
        g["aT"] = aT
        coefw = P["const"].tile([H, NB], F32R, name="coefw")
        nc.sync.dma_start(coefw[:], d_cw[:, :])
        g["coefw"] = coefw
        ctxb = P["const"].tile([H, 1], F32, name="ctxb")
        nc.sync.dma_start(ctxb[:], d_ctx_b[:, :])
        g["ctxb"] = ctxb
        lng = P["const"].tile([128, H], F32, name="lng")
        nc.sync.dma_start(lng[:], d_ln_g[:, :])
        g["lng"] = lng
        lnb = P["const"].tile([128, H], F32, name="lnb")
        nc.sync.dma_start(lnb[:], d_ln_b[:, :])
        g["lnb"] = lnb
        coefb = P["const"].tile([128, NB], F32, name="coefb")
        nc.sync.dma_start(coefb[:], d_cb[:, :])
        g["coefb"] = coefb

        for rep in range(reps):
            for hf in range(2):
                _emit_half(nc, P, g, f"{rep}_{hf}", hf, main_only=main_only)

    nc.compile()
    return nc


_NC = None


def _get_program():
    global _NC
    if _NC is None:
        _NC = _build_program()
    return _NC


def prepare_in_maps(x, context, base_w, base_b, ctx_w, ctx_b, ln_g, ln_b,
                    coeff_w, coeff_b, basis_A, basis_B):
    x = np.asarray(x, np.float32)
    context = np.asarray(context, np.float32)
    base_w = np.asarray(base_w, np.float32)
    base_b = np.asarray(base_b, np.float32)
    ctx_w = np.asarray(ctx_w, np.float32)
    ctx_b = np.asarray(ctx_b, np.float32)
    ln_g = np.asarray(ln_g, np.float32)
    ln_b = np.asarray(ln_b, np.float32)
    coeff_w = np.asarray(coeff_w, np.float32)
    coeff_b = np.asarray(coeff_b, np.float32)
    basis_A = np.asarray(basis_A, np.float32)
    basis_B = np.asarray(basis_B, np.float32)

    # host-side layout prep (transposes / padding / shard)
    xT = np.ascontiguousarray(x.T)
    ctxT = np.ascontiguousarray(context.T)
    ctx_wT = np.ascontiguousarray(ctx_w.T)
    ctx_b_col = np.ascontiguousarray(ctx_b.reshape(H, 1))
    ln_g_bc = np.ascontiguousarray(np.broadcast_to(ln_g[None, :], (128, H)))
    ln_b_bc = np.ascontiguousarray(np.broadcast_to(ln_b[None, :], (128, H)))
    coeff_wT = np.ascontiguousarray(coeff_w.T)
    coeff_b_bc = np.ascontiguousarray(np.broadcast_to(coeff_b[None, :], (128, NB)))
    A_flatT = np.ascontiguousarray(basis_A.transpose(2, 0, 1).reshape(D, NB * RK))

    C_PAD = N_CORES * CS
    bwT = np.zeros((D, C_PAD), np.float32)
    bwT[:, :C_FULL] = base_w.T
    Bm33 = np.zeros((NB * RK + 1, C_PAD), np.float32)
    Bm33[:NB * RK, :C_FULL] = basis_B.transpose(0, 2, 1).reshape(NB * RK, C_FULL)
    Bm33[NB * RK, :C_FULL] = base_b

    rep = {
        "xT": xT, "ctxT": ctxT, "ctx_wT": ctx_wT, "ctx_b_col": ctx_b_col,
        "ln_g_bc": ln_g_bc, "ln_b_bc": ln_b_bc, "coeff_wT": coeff_wT,
        "coeff_b_bc": coeff_b_bc, "A_flatT": A_flatT,
    }
    in_maps = []
    for c in range(N_CORES):
        sl = slice(c * CS, (c + 1) * CS)
        m = dict(rep)
        m["bwT"] = np.ascontiguousarray(bwT[:, sl])
        m["Bm33"] = np.ascontiguousarray(Bm33[:, sl])
        in_maps.append(m)
    return in_maps


def run(in_maps, **spmd_kwargs):
    nc = _get_program()
    res = run_bass_kernel_spmd(nc, in_maps, core_ids=list(range(N_CORES)),
                               **spmd_kwargs)
    out = np.concatenate([res.results[c]["out"] for c in range(N_CORES)], axis=1)
    return np.ascontiguousarray(out[:, :C_FULL]), res


def kernel(**inputs):
    in_maps = prepare_in_maps(**inputs)
    out, _ = run(in_maps)
    return out



# revision 12
# speedup vs baseline: 2.8473x; 2.8473x over previous
# Trainium2 Bass kernel for the MindForge LoRA head problem.
#
# Computation (see reference):
#   h0      = context @ ctx_w.T + ctx_b          (B, H)
#   h       = gelu(LN(h0) * ln_g + ln_b)         (B, H)
#   coeffs  = h @ coeff_w.T + coeff_b            (B, 8)
#   y       = x @ A_flat.T                       (B, 32)   A_flat[(n,r),d] = basis_A[n,r,d]
#   z_br    = sum_n coeffs_bn * y_b(n,r)         (B, 4)
#   u       = coeffs_bn * z_br                   (B, 32)
#   out     = x @ base_w.T + base_b + u @ Bmat   (B, C)    Bmat[(n,r),c] = basis_B[n,c,r]
#
# Distribution: column-parallel over num_classes. Each of the 8 cores computes
# a CS=6400-wide padded shard of the output columns; the tiny ctx-MLP / LoRA
# coefficient pipeline is replicated on every core. base_b is folded into the
# LoRA matmul as a 33rd row of [u | 1] @ [Bmat ; base_b].
#
# v2 layout: single pass over the class shard with x^T fully resident in SBUF
# (bf16), so base_w streams through exactly once per rep. x and base_w are
# bf16 (same 1 cycle/row PE rate as fp32r, half the DMA/SBUF); everything else
# stays fp32/fp32r. Weight panels are pre-tiled on the host into a k-major
# [128, KT*CS] DRAM layout so each panel is a single 2 MB dma_start with 16 KB
# contiguous per-partition lines. LayerNorm/GELU run batched over all 2048
# samples in one set of DVE/ACT ops.

import numpy as np
import ml_dtypes
from contextlib import ExitStack

import concourse.bass as bass
import concourse.tile as tile
from concourse import bacc, mybir
from concourse.bass_utils import run_bass_kernel_spmd
from concourse.masks import make_identity

F32 = mybir.dt.float32
F32R = mybir.dt.float32r
BF16 = mybir.dt.bfloat16
AF = mybir.ActivationFunctionType
AX = mybir.AxisListType
NPBF16 = ml_dtypes.bfloat16

D = 2048          # d_model
B = 2048          # batch
C_FULL = 50257    # num_classes
NB = 8            # n_basis
RK = 4            # rank
H = 128           # hidden
N_CORES = 8
CS = 6400         # per-core padded class shard (8*6400 = 51200 >= 50257)
LN_EPS = 1e-5

KT = D // 128            # 16 k-tiles
BT = B // 128            # 16 b-tiles
NT = 512                 # phase-A stream chunk
C_TILES = [512] * 12 + [256]   # sums to 6400
assert sum(C_TILES) == CS
# panel ci starts at byte-free-offset OFF[ci] in the pre-tiled bwp tensor
OFF = [0]
for _w in C_TILES:
    OFF.append(OFF[-1] + KT * _w)
assert OFF[-1] == KT * CS


def _emit_rep(nc, P, g, u):
    d_xt, d_ctxT, d_bwp, d_Bm, d_out = (g["d_xt"], g["d_ctxT"], g["d_bwp"],
                                        g["d_Bm"], g["d_out"])
    ident, cwT, aT, coefw, ctxb, lng, lnb, coefb = (
        g["ident"], g["cwT"], g["aT"], g["coefw"], g["ctxb"], g["lng"],
        g["lnb"], g["coefb"])

    # resident x^T (k-major, full batch) — issued first so it overlaps phase A
    xt = P["xpool"].tile([128, KT * B], BF16, tag="xt", name=f"xt_{u}")
    nc.sync.dma_start(xt[:], d_xt[:, :])

    # =========== phase A: coefficient pipeline (replicated) ===========
    # A1: h0^T = ctx_w @ context^T (H on partitions), +ctx_b, transpose -> h0
    h0 = P["sbA"].tile([128, B], F32, tag="h0", name=f"h0_{u}")
    for bc in range(B // NT):
        cx = P["cstr"].tile([128, KT * NT], BF16, tag="cx", name=f"cx_{u}_{bc}")
        nc.sync.dma_start(cx[:], d_ctxT[:, bc * KT * NT:(bc + 1) * KT * NT])
        acc = P["psA"].tile([128, NT], F32, tag="acc", name=f"h0T_ps_{u}_{bc}")
        for k in range(KT):
            nc.tensor.matmul(acc[:], cwT[:, k * H:(k + 1) * H],
                             cx[:, k * NT:(k + 1) * NT],
                             start=(k == 0), stop=(k == KT - 1))
        h0T = P["sbB"].tile([128, NT], F32, tag="h0T", name=f"h0T_{u}_{bc}")
        nc.scalar.activation(h0T[:], acc[:], AF.Identity, bias=ctxb[:])
        for j in range(NT // 128):
            t = bc * (NT // 128) + j
            tr = P["psA"].tile([128, 128], F32, tag="tr", name=f"h0tr_{u}_{t}")
            nc.tensor.transpose(tr[:], h0T[:, j * 128:(j + 1) * 128], ident[:])
            nc.vector.tensor_copy(h0[:, t * 128:(t + 1) * 128], tr[:])

    # A4: y^T = A_flat @ x^T, transpose to y (batch on partitions)
    ysb = P["sbA"].tile([128, BT * 32], F32, tag="ysb", name=f"ysb_{u}")
    for bc in range(B // NT):
        acc = P["psA"].tile([32, NT], F32, tag="acc", name=f"yT_ps_{u}_{bc}")
        for k in range(KT):
            nc.tensor.matmul(acc[:], aT[:, k * 32:(k + 1) * 32],
                             xt[:, k * B + bc * NT:k * B + (bc + 1) * NT],
                             start=(k == 0), stop=(k == KT - 1))
        yT = P["sbB"].tile([32, NT], F32, tag="yT", name=f"yT_{u}_{bc}")
        nc.vector.tensor_copy(yT[:], acc[:])
        for j in range(NT // 128):
            t = bc * (NT // 128) + j
            tr = P["psA"].tile([128, 32], F32, tag="tr", name=f"ytr_{u}_{t}")
            nc.tensor.transpose(tr[:], yT[:, j * 128:(j + 1) * 128],
                                ident[:32, :32])
            nc.vector.tensor_copy(ysb[:, t * 32:(t + 1) * 32], tr[:])

    # A2: batched LayerNorm + GELU over all BT tiles at once
    sq = P["sbA"].tile([128, B], F32, tag="sq", name=f"sq_{u}")
    mu = P["small"].tile([128, BT], F32, tag="mu", name=f"mu_{u}")
    s2 = P["small"].tile([128, BT], F32, tag="s2", name=f"s2_{u}")
    h3 = h0[:].rearrange("p (t h) -> p t h", h=H)
    nc.vector.reduce_sum(mu[:], h3, axis=AX.X)
    nc.vector.tensor_scalar_mul(mu[:], mu[:], 1.0 / H)
    nc.scalar.activation(sq[:], h0[:], AF.Square)
    nc.vector.reduce_sum(s2[:], sq[:].rearrange("p (t h) -> p t h", h=H),
                         axis=AX.X)
    nc.vector.tensor_scalar_mul(s2[:], s2[:], 1.0 / H)
    var = P["small"].tile([128, BT], F32, tag="var", name=f"var_{u}")
    nc.vector.tensor_mul(var[:], mu[:], mu[:])
    nc.vector.tensor_sub(var[:], s2[:], var[:])
    nc.vector.tensor_scalar_add(var[:], var[:], LN_EPS)
    std = P["small"].tile([128, BT], F32, tag="std", name=f"std_{u}")
    nc.scalar.sqrt(std[:], var[:])
    rstd = P["small"].tile([128, BT], F32, tag="rstd", name=f"rstd_{u}")
    nc.vector.reciprocal(rstd[:], std[:])
    nmu = P["small"].tile([128, BT], F32, tag="nmu", name=f"nmu_{u}")
    nc.vector.tensor_mul(nmu[:], mu[:], rstd[:])
    nc.vector.tensor_scalar_mul(nmu[:], nmu[:], -1.0)
    # h0 = (h0 * rstd - mu*rstd) * ln_g + ln_b, then gelu -> sq
    nc.vector.tensor_mul(h3, h3, rstd[:].unsqueeze(2).broadcast_to((128, BT, H)))
    nc.vector.tensor_add(h3, h3, nmu[:].unsqueeze(2).broadcast_to((128, BT, H)))
    nc.vector.tensor_mul(h3, h3, lng[:].unsqueeze(1).broadcast_to((128, BT, H)))
    nc.vector.tensor_add(h3, h3, lnb[:].unsqueeze(1).broadcast_to((128, BT, H)))
    nc.scalar.activation(sq[:], h0[:], AF.Gelu)   # sq now holds h

    # A3: transpose h -> hT, coeffs = h @ coeff_w^T + coeff_b
    hT = P["sbA"].tile([128, B], F32R, tag="hT", name=f"hT_{u}")
    cfs = P["sbA"].tile([128, BT * NB], F32, tag="cfs", name=f"cfs_{u}")
    for t in range(BT):
        tr = P["psA"].tile([128, 128], F32, tag="tr", name=f"htr_{u}_{t}")
        nc.tensor.transpose(tr[:], sq[:, t * 128:(t + 1) * 128], ident[:])
        nc.vector.tensor_copy(hT[:, t * 128:(t + 1) * 128], tr[:])
        cf = P["psA"].tile([128, NB], F32, tag="tr", name=f"cf_ps_{u}_{t}")
        nc.tensor.matmul(cf[:], hT[:, t * 128:(t + 1) * 128], coefw[:],
                         start=True, stop=True)
        nc.vector.tensor_add(cfs[:, t * NB:(t + 1) * NB], cf[:], coefb[:])

    # A5: z = sum_n coeffs*y ; u = coeffs (x) z ; uT33 = [u | 1]^T
    # bf16 so the K=33 LoRA matmul matches the dtype of the bf16 k-tile
    # matmuls it shares a PSUM accumulation group with.
    uT33 = P["sbA"].tile([NB * RK + 1, B], BF16, tag="uT33", name=f"uT33_{u}")
    for t in range(BT):
        yb = ysb[:, t * 32:(t + 1) * 32]
        cb = cfs[:, t * NB:(t + 1) * NB]
        prod = P["small"].tile([128, 32], F32, tag="prod", name=f"prod_{u}_{t}")
        # prod stored r-major: prod[p, r*8+n] = y[p, n*4+r] * coeffs[p, n]
        nc.vector.tensor_mul(
            prod[:].rearrange("p (r n) -> p r n", n=NB),
            yb.rearrange("p (n r) -> p r n", r=RK),
            cb.unsqueeze(1).broadcast_to((128, RK, NB)))
        z = P["small"].tile([128, RK], F32, tag="z", name=f"z_{u}_{t}")
        nc.vector.reduce_sum(z[:], prod[:].rearrange("p (r n) -> p r n", n=NB),
                             axis=AX.X)
        ut = P["small"].tile([128, NB * RK + 1], F32, tag="u", name=f"u_{u}_{t}")
        nc.vector.tensor_mul(
            ut[:, :NB * RK].rearrange("p (n r) -> p n r", r=RK),
            cb.unsqueeze(2).broadcast_to((128, NB, RK)),
            z[:].unsqueeze(1).broadcast_to((128, NB, RK)))
        nc.gpsimd.memset(ut[:, NB * RK:NB * RK + 1], 1.0)
        tr = P["psA"].tile([NB * RK + 1, 128], F32, tag="tr", name=f"utr_{u}_{t}")
        nc.tensor.transpose(tr[:], ut[:], ident[:])
        nc.vector.tensor_copy(uT33[:, t * 128:(t + 1) * 128], tr[:])

    # =========== phase B: main column-parallel matmul ===========
    c0 = 0
    for ci, W in enumerate(C_TILES):
        panel = P["bwp"].tile([128, KT * W], BF16, tag="bw", name=f"bw_{u}_{ci}")
        nc.sync.dma_start(panel[:], d_bwp[:, OFF[ci]:OFF[ci + 1]])
        bx = P["bxp"].tile([NB * RK + 1, W], BF16, tag="bx", name=f"bx_{u}_{ci}")
        nc.sync.dma_start(bx[:], d_Bm[:, c0:c0 + W])
        for b in range(BT):
            po = P["psM"].tile([128, W], F32, tag="out", name=f"po_{u}_{ci}_{b}")
            for k in range(KT):
                nc.tensor.matmul(
                    po[:],
                    xt[:, k * B + b * 128:k * B + (b + 1) * 128],
                    panel[:, k * W:(k + 1) * W],
                    start=(k == 0), stop=False)
            nc.tensor.matmul(po[:], uT33[:, b * 128:(b + 1) * 128],
                             bx[:], start=False, stop=True)
            ot = P["outp"].tile([128, W], F32, tag="ot", name=f"ot_{u}_{ci}_{b}")
            nc.vector.tensor_copy(ot[:], po[:])
            nc.sync.dma_start(
                d_out[b * 128:(b + 1) * 128, c0:c0 + W], ot[:])
        c0 += W


def _build_program(reps=1):
    nc = bacc.Bacc("TRN2", target_bir_lowering=False, debug=False,
                   num_devices=N_CORES)

    g = {}
    # DRAM I/O (per-core shapes)
    g["d_xt"] = nc.dram_tensor("xt_t", [128, KT * B], BF16,
                               kind="ExternalInput").ap()
    g["d_ctxT"] = nc.dram_tensor("ctx_t", [128, KT * B], BF16,
                                 kind="ExternalInput").ap()
    d_cwt = nc.dram_tensor("cwt_t", [128, KT * H], BF16,
                           kind="ExternalInput").ap()
    d_ctx_b = nc.dram_tensor("ctx_b_col", [H, 1], F32, kind="ExternalInput").ap()
    d_ln_g = nc.dram_tensor("ln_g_bc", [128, H], F32, kind="ExternalInput").ap()
    d_ln_b = nc.dram_tensor("ln_b_bc", [128, H], F32, kind="ExternalInput").ap()
    d_cw = nc.dram_tensor("coeff_wT", [H, NB], F32R, kind="ExternalInput").ap()
    d_cb = nc.dram_tensor("coeff_b_bc", [128, NB], F32,
                          kind="ExternalInput").ap()
    d_at = nc.dram_tensor("aT_t", [128, KT * NB * RK], BF16,
                          kind="ExternalInput").ap()
    g["d_bwp"] = nc.dram_tensor("bwp", [128, KT * CS], BF16,
                                kind="ExternalInput").ap()
    g["d_Bm"] = nc.dram_tensor("Bm33", [NB * RK + 1, CS], BF16,
                               kind="ExternalInput").ap()
    g["d_out"] = nc.dram_tensor("out", [B, CS], F32, kind="ExternalOutput").ap()

    with tile.TileContext(nc) as tc, ExitStack() as ctx:
        P = {}
        P["const"] = ctx.enter_context(tc.tile_pool(name="const", bufs=1))
        P["sbA"] = ctx.enter_context(tc.tile_pool(name="sbA", bufs=1))
        P["sbB"] = ctx.enter_context(tc.tile_pool(name="sbB", bufs=2))
        P["small"] = ctx.enter_context(tc.tile_pool(name="small", bufs=4))
        P["cstr"] = ctx.enter_context(tc.tile_pool(name="cstr", bufs=2))
        P["xpool"] = ctx.enter_context(tc.tile_pool(name="xpool", bufs=1))
        P["bwp"] = ctx.enter_context(tc.tile_pool(name="bwp", bufs=2))
        P["bxp"] = ctx.enter_context(tc.tile_pool(name="bxp", bufs=2))
        P["outp"] = ctx.enter_context(tc.tile_pool(name="outp", bufs=8))
        P["psA"] = ctx.enter_context(tc.tile_pool(name="psA", bufs=2, space="PSUM"))
        P["psM"] = ctx.enter_context(tc.tile_pool(name="psM", bufs=4, space="PSUM"))

        # ---- constants / replicated small tensors ----
        ident = P["const"].tile([128, 128], F32, name="ident")
        make_identity(nc, ident[:])
        g["ident"] = ident
        cwT = P["const"].tile([128, KT * H], BF16, name="cwT")
        nc.sync.dma_start(cwT[:], d_cwt[:, :])
        g["cwT"] = cwT
        aT = P["const"].tile([128, KT * NB * RK], BF16, name="aT")
        nc.sync.dma_start(aT[:], d_at[:, :])
        g["aT"] = aT
        coefw = P["const"].tile([H, NB], F32R, name="coefw")
        nc.sync.dma_start(coefw[:], d_cw[:, :])
        g["coefw"] = coefw
        ctxb = P["const"].tile([H, 1], F32, name="ctxb")
        nc.sync.dma_start(ctxb[:], d_ctx_b[:, :])
        g["ctxb"] = ctxb
        lng = P["const"].tile([128, H], F32, name="lng")
        nc.sync.dma_start(lng[:], d_ln_g[:, :])
        g["lng"] = lng
        lnb = P["const"].tile([128, H], F32, name="lnb")
        nc.sync.dma_start(lnb[:], d_ln_b[:, :])
        g["lnb"] = lnb
        coefb = P["const"].tile([128, NB], F32, name="coefb")
        nc.sync.dma_start(coefb[:], d_cb[:, :])
        g["coefb"] = coefb

        for rep in range(reps):
            _emit_rep(nc, P, g, f"{rep}")

    nc.compile()
    return nc


_NC = None


def _get_program():
    global _NC
    if _NC is None:
        _NC = _build_program()
    return _NC


def _ktile(a, width):
    """[D, width] -> [128, KT*width] k-major per-partition layout."""
    return np.ascontiguousarray(
        a.reshape(KT, 128, width).transpose(1, 0, 2).reshape(128, KT * width))


def prepare_in_maps(x, context, base_w, base_b, ctx_w, ctx_b, ln_g, ln_b,
                    coeff_w, coeff_b, basis_A, basis_B):
    x = np.asarray(x, np.float32)
    context = np.asarray(context, np.float32)
    base_w = np.asarray(base_w, np.float32)
    base_b = np.asarray(base_b, np.float32)
    ctx_w = np.asarray(ctx_w, np.float32)
    ctx_b = np.asarray(ctx_b, np.float32)
    ln_g = np.asarray(ln_g, np.float32)
    ln_b = np.asarray(ln_b, np.float32)
    coeff_w = np.asarray(coeff_w, np.float32)
    coeff_b = np.asarray(coeff_b, np.float32)
    basis_A = np.asarray(basis_A, np.float32)
    basis_B = np.asarray(basis_B, np.float32)

    xt_t = _ktile(np.ascontiguousarray(x.T), B).astype(NPBF16)
    # ctx^T chunk-major: [p, bc*(KT*NT) + k*NT + j] = context[bc*NT+j, k*128+p]
    ctx_t = np.ascontiguousarray(
        context.T.reshape(KT, 128, B // NT, NT).transpose(1, 2, 0, 3)
        .reshape(128, KT * B)).astype(NPBF16)
    cwt_t = _ktile(np.ascontiguousarray(ctx_w.T), H).astype(NPBF16)
    ctx_b_col = np.ascontiguousarray(ctx_b.reshape(H, 1))
    ln_g_bc = np.ascontiguousarray(np.broadcast_to(ln_g[None, :], (128, H)))
    ln_b_bc = np.ascontiguousarray(np.broadcast_to(ln_b[None, :], (128, H)))
    coeff_wT = np.ascontiguousarray(coeff_w.T)
    coeff_b_bc = np.ascontiguousarray(np.broadcast_to(coeff_b[None, :], (128, NB)))
    A_flatT = np.ascontiguousarray(basis_A.transpose(2, 0, 1).reshape(D, NB * RK))
    aT_t = _ktile(A_flatT, NB * RK).astype(NPBF16)

    C_PAD = N_CORES * CS
    bwT = np.zeros((D, C_PAD), np.float32)
    bwT[:, :C_FULL] = base_w.T
    Bm33 = np.zeros((NB * RK + 1, C_PAD), np.float32)
    Bm33[:NB * RK, :C_FULL] = basis_B.transpose(0, 2, 1).reshape(NB * RK, C_FULL)
    Bm33[NB * RK, :C_FULL] = base_b

    rep = {
        "xt_t": xt_t, "ctx_t": ctx_t, "cwt_t": cwt_t, "ctx_b_col": ctx_b_col,
        "ln_g_bc": ln_g_bc, "ln_b_bc": ln_b_bc, "coeff_wT": coeff_wT,
        "coeff_b_bc": coeff_b_bc, "aT_t": aT_t,
    }
    in_maps = []
    for c in range(N_CORES):
        sl = slice(c * CS, (c + 1) * CS)
        shard = bwT[:, sl]
        parts = []
        c0 = 0
        for W in C_TILES:
            parts.append(_ktile(np.ascontiguousarray(shard[:, c0:c0 + W]), W))
            c0 += W
        m = dict(rep)
        m["bwp"] = np.concatenate(parts, axis=1).astype(NPBF16)
        m["Bm33"] = np.ascontiguousarray(Bm33[:, sl]).astype(NPBF16)
        in_maps.append(m)
    return in_maps


def run(in_maps, **spmd_kwargs):
    nc = _get_program()
    res = run_bass_kernel_spmd(nc, in_maps, core_ids=list(range(N_CORES)),
                               **spmd_kwargs)
    out = np.concatenate([res.results[c]["out"] for c in range(N_CORES)], axis=1)
    return np.ascontiguousarray(out[:, :C_FULL]), res


def kernel(**inputs):
    in_maps = prepare_in_maps(**inputs)
    out, _ = run(in_maps)
    return out


# revision 13
# speedup vs baseline: 3.1807x; 1.1171x over previous
# Trainium2 Bass kernel for the MindForge LoRA head problem.
#
# Computation (see reference):
#   h0      = context @ ctx_w.T + ctx_b          (B, H)
#   h       = gelu(LN(h0) * ln_g + ln_b)         (B, H)
#   coeffs  = h @ coeff_w.T + coeff_b            (B, 8)
#   y       = x @ A_flat.T                       (B, 32)   A_flat[(n,r),d] = basis_A[n,r,d]
#   z_br    = sum_n coeffs_bn * y_b(n,r)         (B, 4)
#   u       = coeffs_bn * z_br                   (B, 32)
#   out     = x @ base_w.T + base_b + u @ Bmat   (B, C)    Bmat[(n,r),c] = basis_B[n,c,r]
#
# Distribution: column-parallel over num_classes. Each of the 8 cores computes
# a CS=6400-wide padded shard of the output columns; the tiny ctx-MLP / LoRA
# coefficient pipeline is replicated on every core. base_b is folded into the
# LoRA matmul as a 33rd row of [u | 1] @ [Bmat ; base_b].
#
# v2 layout: single pass over the class shard with x^T fully resident in SBUF
# (bf16), so base_w streams through exactly once per rep. x and base_w are
# bf16 (same 1 cycle/row PE rate as fp32r, half the DMA/SBUF); everything else
# stays fp32/fp32r. Weight panels are pre-tiled on the host into a k-major
# [128, KT*CS] DRAM layout so each panel is a single 2 MB dma_start with 16 KB
# contiguous per-partition lines. LayerNorm/GELU run batched over all 2048
# samples in one set of DVE/ACT ops.

import numpy as np
import ml_dtypes
from contextlib import ExitStack

import concourse.bass as bass
import concourse.tile as tile
from concourse import bacc, mybir
from concourse.bass_utils import run_bass_kernel_spmd
from concourse.masks import make_identity

F32 = mybir.dt.float32
F32R = mybir.dt.float32r
BF16 = mybir.dt.bfloat16
AF = mybir.ActivationFunctionType
AX = mybir.AxisListType
NPBF16 = ml_dtypes.bfloat16

D = 2048          # d_model
B = 2048          # batch
C_FULL = 50257    # num_classes
NB = 8            # n_basis
RK = 4            # rank
H = 128           # hidden
N_CORES = 8
CS = 6284         # per-core padded class shard (8*6284 = 50272 >= 50257)
LN_EPS = 1e-5

KT = D // 128            # 16 k-tiles
BT = B // 128            # 16 b-tiles
NT = 512                 # phase-A stream chunk
C_TILES = [512] * 12 + [140]   # sums to 6284
assert sum(C_TILES) == CS
# panel ci starts at byte-free-offset OFF[ci] in the pre-tiled bwp tensor
OFF = [0]
for _w in C_TILES:
    OFF.append(OFF[-1] + KT * _w)
assert OFF[-1] == KT * CS


def _emit_rep(nc, P, g, u):
    d_xt, d_ctxT, d_bwp, d_Bm, d_out = (g["d_xt"], g["d_ctxT"], g["d_bwp"],
                                        g["d_Bm"], g["d_out"])
    ident, cwT, aT, coefw, ctxb, lng, lnb, coefb = (
        g["ident"], g["cwT"], g["aT"], g["coefw"], g["ctxb"], g["lng"],
        g["lnb"], g["coefb"])

    # resident x^T (k-major, full batch) — issued first so it overlaps phase A
    xt = P["xpool"].tile([128, KT * B], BF16, tag="xt", name=f"xt_{u}")
    nc.sync.dma_start(xt[:], d_xt[:, :])

    # =========== phase A: coefficient pipeline (replicated) ===========
    # A1: h0^T = ctx_w @ context^T (H on partitions), +ctx_b, transpose -> h0
    h0 = P["sbA"].tile([128, B], F32, tag="h0", name=f"h0_{u}")
    for bc in range(B // NT):
        cx = P["cstr"].tile([128, KT * NT], BF16, tag="cx", name=f"cx_{u}_{bc}")
        nc.sync.dma_start(cx[:], d_ctxT[:, bc * KT * NT:(bc + 1) * KT * NT])
        acc = P["psA"].tile([128, NT], F32, tag="acc", name=f"h0T_ps_{u}_{bc}")
        for k in range(KT):
            nc.tensor.matmul(acc[:], cwT[:, k * H:(k + 1) * H],
                             cx[:, k * NT:(k + 1) * NT],
                             start=(k == 0), stop=(k == KT - 1))
        h0T = P["sbB"].tile([128, NT], F32, tag="h0T", name=f"h0T_{u}_{bc}")
        nc.scalar.activation(h0T[:], acc[:], AF.Identity, bias=ctxb[:])
        for j in range(NT // 128):
            t = bc * (NT // 128) + j
            tr = P["psA"].tile([128, 128], F32, tag="tr", name=f"h0tr_{u}_{t}")
            nc.tensor.transpose(tr[:], h0T[:, j * 128:(j + 1) * 128], ident[:])
            nc.vector.tensor_copy(h0[:, t * 128:(t + 1) * 128], tr[:])

    # A4: y^T = A_flat @ x^T, transpose to y (batch on partitions)
    ysb = P["sbA"].tile([128, BT * 32], F32, tag="ysb", name=f"ysb_{u}")
    for bc in range(B // NT):
        acc = P["psA"].tile([32, NT], F32, tag="acc", name=f"yT_ps_{u}_{bc}")
        for k in range(KT):
            nc.tensor.matmul(acc[:], aT[:, k * 32:(k + 1) * 32],
                             xt[:, k * B + bc * NT:k * B + (bc + 1) * NT],
                             start=(k == 0), stop=(k == KT - 1))
        yT = P["sbB"].tile([32, NT], F32, tag="yT", name=f"yT_{u}_{bc}")
        nc.vector.tensor_copy(yT[:], acc[:])
        for j in range(NT // 128):
            t = bc * (NT // 128) + j
            tr = P["psA"].tile([128, 32], F32, tag="tr", name=f"ytr_{u}_{t}")
            nc.tensor.transpose(tr[:], yT[:, j * 128:(j + 1) * 128],
                                ident[:32, :32])
            nc.vector.tensor_copy(ysb[:, t * 32:(t + 1) * 32], tr[:])

    # A2: batched LayerNorm + GELU over all BT tiles at once
    sq = P["sbA"].tile([128, B], F32, tag="sq", name=f"sq_{u}")
    mu = P["small"].tile([128, BT], F32, tag="mu", name=f"mu_{u}")
    s2 = P["small"].tile([128, BT], F32, tag="s2", name=f"s2_{u}")
    h3 = h0[:].rearrange("p (t h) -> p t h", h=H)
    nc.vector.reduce_sum(mu[:], h3, axis=AX.X)
    nc.vector.tensor_scalar_mul(mu[:], mu[:], 1.0 / H)
    nc.scalar.activation(sq[:], h0[:], AF.Square)
    nc.vector.reduce_sum(s2[:], sq[:].rearrange("p (t h) -> p t h", h=H),
                         axis=AX.X)
    nc.vector.tensor_scalar_mul(s2[:], s2[:], 1.0 / H)
    var = P["small"].tile([128, BT], F32, tag="var", name=f"var_{u}")
    nc.vector.tensor_mul(var[:], mu[:], mu[:])
    nc.vector.tensor_sub(var[:], s2[:], var[:])
    nc.vector.tensor_scalar_add(var[:], var[:], LN_EPS)
    std = P["small"].tile([128, BT], F32, tag="std", name=f"std_{u}")
    nc.scalar.sqrt(std[:], var[:])
    rstd = P["small"].tile([128, BT], F32, tag="rstd", name=f"rstd_{u}")
    nc.vector.reciprocal(rstd[:], std[:])
    nmu = P["small"].tile([128, BT], F32, tag="nmu", name=f"nmu_{u}")
    nc.vector.tensor_mul(nmu[:], mu[:], rstd[:])
    nc.vector.tensor_scalar_mul(nmu[:], nmu[:], -1.0)
    # h0 = (h0 * rstd - mu*rstd) * ln_g + ln_b, then gelu -> sq
    nc.vector.tensor_mul(h3, h3, rstd[:].unsqueeze(2).broadcast_to((128, BT, H)))
    nc.vector.tensor_add(h3, h3, nmu[:].unsqueeze(2).broadcast_to((128, BT, H)))
    nc.vector.tensor_mul(h3, h3, lng[:].unsqueeze(1).broadcast_to((128, BT, H)))
    nc.vector.tensor_add(h3, h3, lnb[:].unsqueeze(1).broadcast_to((128, BT, H)))
    nc.scalar.activation(sq[:], h0[:], AF.Gelu)   # sq now holds h

    # A3: transpose h -> hT, coeffs = h @ coeff_w^T + coeff_b
    hT = P["sbA"].tile([128, B], F32R, tag="hT", name=f"hT_{u}")
    cfs = P["sbA"].tile([128, BT * NB], F32, tag="cfs", name=f"cfs_{u}")
    for t in range(BT):
        tr = P["psA"].tile([128, 128], F32, tag="tr", name=f"htr_{u}_{t}")
        nc.tensor.transpose(tr[:], sq[:, t * 128:(t + 1) * 128], ident[:])
        nc.vector.tensor_copy(hT[:, t * 128:(t + 1) * 128], tr[:])
        cf = P["psA"].tile([128, NB], F32, tag="tr", name=f"cf_ps_{u}_{t}")
        nc.tensor.matmul(cf[:], hT[:, t * 128:(t + 1) * 128], coefw[:],
                         start=True, stop=True)
        nc.vector.tensor_add(cfs[:, t * NB:(t + 1) * NB], cf[:], coefb[:])

    # A5: z = sum_n coeffs*y ; u = coeffs (x) z ; uT33 = [u | 1]^T
    # bf16 so the K=33 LoRA matmul matches the dtype of the bf16 k-tile
    # matmuls it shares a PSUM accumulation group with.
    uT33 = P["sbA"].tile([NB * RK + 1, B], BF16, tag="uT33", name=f"uT33_{u}")
    for t in range(BT):
        yb = ysb[:, t * 32:(t + 1) * 32]
        cb = cfs[:, t * NB:(t + 1) * NB]
        prod = P["small"].tile([128, 32], F32, tag="prod", name=f"prod_{u}_{t}")
        # prod stored r-major: prod[p, r*8+n] = y[p, n*4+r] * coeffs[p, n]
        nc.vector.tensor_mul(
            prod[:].rearrange("p (r n) -> p r n", n=NB),
            yb.rearrange("p (n r) -> p r n", r=RK),
            cb.unsqueeze(1).broadcast_to((128, RK, NB)))
        z = P["small"].tile([128, RK], F32, tag="z", name=f"z_{u}_{t}")
        nc.vector.reduce_sum(z[:], prod[:].rearrange("p (r n) -> p r n", n=NB),
                             axis=AX.X)
        ut = P["small"].tile([128, NB * RK + 1], F32, tag="u", name=f"u_{u}_{t}")
        nc.vector.tensor_mul(
            ut[:, :NB * RK].rearrange("p (n r) -> p n r", r=RK),
            cb.unsqueeze(2).broadcast_to((128, NB, RK)),
            z[:].unsqueeze(1).broadcast_to((128, NB, RK)))
        nc.gpsimd.memset(ut[:, NB * RK:NB * RK + 1], 1.0)
        tr = P["psA"].tile([NB * RK + 1, 128], F32, tag="tr", name=f"utr_{u}_{t}")
        nc.tensor.transpose(tr[:], ut[:], ident[:])
        nc.vector.tensor_copy(uT33[:, t * 128:(t + 1) * 128], tr[:])

    # =========== phase B: main column-parallel matmul ===========
    c0 = 0
    for ci, W in enumerate(C_TILES):
        panel = P["bwp"].tile([128, KT * W], BF16, tag="bw", name=f"bw_{u}_{ci}")
        nc.sync.dma_start(panel[:], d_bwp[:, OFF[ci]:OFF[ci + 1]])
        bx = P["bxp"].tile([NB * RK + 1, W], BF16, tag="bx", name=f"bx_{u}_{ci}")
        nc.sync.dma_start(bx[:], d_Bm[:, c0:c0 + W])
        for b in range(BT):
            po = P["psM"].tile([128, W], F32, tag="out", name=f"po_{u}_{ci}_{b}")
            for k in range(KT):
                nc.tensor.matmul(
                    po[:],
                    xt[:, k * B + b * 128:k * B + (b + 1) * 128],
                    panel[:, k * W:(k + 1) * W],
                    start=(k == 0), stop=False)
            nc.tensor.matmul(po[:], uT33[:, b * 128:(b + 1) * 128],
                             bx[:], start=False, stop=True)
            ot = P["outp"].tile([128, W], F32, tag="ot", name=f"ot_{u}_{ci}_{b}")
            nc.vector.tensor_copy(ot[:], po[:])
            nc.sync.dma_start(
                d_out[b * 128:(b + 1) * 128, c0:c0 + W], ot[:])
        c0 += W


def _build_program(reps=1):
    nc = bacc.Bacc("TRN2", target_bir_lowering=False, debug=False,
                   num_devices=N_CORES)

    g = {}
    # DRAM I/O (per-core shapes)
    g["d_xt"] = nc.dram_tensor("xt_t", [128, KT * B], BF16,
                               kind="ExternalInput").ap()
    g["d_ctxT"] = nc.dram_tensor("ctx_t", [128, KT * B], BF16,
                                 kind="ExternalInput").ap()
    d_cwt = nc.dram_tensor("cwt_t", [128, KT * H], BF16,
                           kind="ExternalInput").ap()
    d_ctx_b = nc.dram_tensor("ctx_b_col", [H, 1], F32, kind="ExternalInput").ap()
    d_ln_g = nc.dram_tensor("ln_g_bc", [128, H], F32, kind="ExternalInput").ap()
    d_ln_b = nc.dram_tensor("ln_b_bc", [128, H], F32, kind="ExternalInput").ap()
    d_cw = nc.dram_tensor("coeff_wT", [H, NB], F32R, kind="ExternalInput").ap()
    d_cb = nc.dram_tensor("coeff_b_bc", [128, NB], F32,
                          kind="ExternalInput").ap()
    d_at = nc.dram_tensor("aT_t", [128, KT * NB * RK], BF16,
                          kind="ExternalInput").ap()
    g["d_bwp"] = nc.dram_tensor("bwp", [128, KT * CS], BF16,
                                kind="ExternalInput").ap()
    g["d_Bm"] = nc.dram_tensor("Bm33", [NB * RK + 1, CS], BF16,
                               kind="ExternalInput").ap()
    g["d_out"] = nc.dram_tensor("out", [B, CS], F32, kind="ExternalOutput").ap()

    with tile.TileContext(nc) as tc, ExitStack() as ctx:
        P = {}
        P["const"] = ctx.enter_context(tc.tile_pool(name="const", bufs=1))
        P["sbA"] = ctx.enter_context(tc.tile_pool(name="sbA", bufs=1))
        P["sbB"] = ctx.enter_context(tc.tile_pool(name="sbB", bufs=2))
        P["small"] = ctx.enter_context(tc.tile_pool(name="small", bufs=4))
        P["cstr"] = ctx.enter_context(tc.tile_pool(name="cstr", bufs=2))
        P["xpool"] = ctx.enter_context(tc.tile_pool(name="xpool", bufs=1))
        P["bwp"] = ctx.enter_context(tc.tile_pool(name="bwp", bufs=2))
        P["bxp"] = ctx.enter_context(tc.tile_pool(name="bxp", bufs=2))
        P["outp"] = ctx.enter_context(tc.tile_pool(name="outp", bufs=8))
        P["psA"] = ctx.enter_context(tc.tile_pool(name="psA", bufs=2, space="PSUM"))
        P["psM"] = ctx.enter_context(tc.tile_pool(name="psM", bufs=4, space="PSUM"))

        # ---- constants / replicated small tensors ----
        ident = P["const"].tile([128, 128], F32, name="ident")
        make_identity(nc, ident[:])
        g["ident"] = ident
        cwT = P["const"].tile([128, KT * H], BF16, name="cwT")
        nc.sync.dma_start(cwT[:], d_cwt[:, :])
        g["cwT"] = cwT
        aT = P["const"].tile([128, KT * NB * RK], BF16, name="aT")
        nc.sync.dma_start(aT[:], d_at[:, :])
        g["aT"] = aT
        coefw = P["const"].tile([H, NB], F32R, name="coefw")
        nc.sync.dma_start(coefw[:], d_cw[:, :])
        g["coefw"] = coefw
        ctxb = P["const"].tile([H, 1], F32, name="ctxb")
        nc.sync.dma_start(ctxb[:], d_ctx_b[:, :])
        g["ctxb"] = ctxb
        lng = P["const"].tile([128, H], F32, name="lng")
        nc.sync.dma_start(lng[:], d_ln_g[:, :])
        g["lng"] = lng
        lnb = P["const"].tile([128, H], F32, name="lnb")
        nc.sync.dma_start(lnb[:], d_ln_b[:, :])
        g["lnb"] = lnb
        coefb = P["const"].tile([128, NB], F32, name="coefb")
        nc.sync.dma_start(coefb[:], d_cb[:, :])
        g["coefb"] = coefb

        for rep in range(reps):
            _emit_rep(nc, P, g, f"{rep}")

    nc.compile()
    return nc


_NC = None


def _get_program():
    global _NC
    if _NC is None:
        _NC = _build_program()
    return _NC


def _ktile(a, width):
    """[D, width] -> [128, KT*width] k-major per-partition layout."""
    return np.ascontiguousarray(
        a.reshape(KT, 128, width).transpose(1, 0, 2).reshape(128, KT * width))


def prepare_in_maps(x, context, base_w, base_b, ctx_w, ctx_b, ln_g, ln_b,
                    coeff_w, coeff_b, basis_A, basis_B):
    x = np.asarray(x, np.float32)
    context = np.asarray(context, np.float32)
    base_w = np.asarray(base_w, np.float32)
    base_b = np.asarray(base_b, np.float32)
    ctx_w = np.asarray(ctx_w, np.float32)
    ctx_b = np.asarray(ctx_b, np.float32)
    ln_g = np.asarray(ln_g, np.float32)
    ln_b = np.asarray(ln_b, np.float32)
    coeff_w = np.asarray(coeff_w, np.float32)
    coeff_b = np.asarray(coeff_b, np.float32)
    basis_A = np.asarray(basis_A, np.float32)
    basis_B = np.asarray(basis_B, np.float32)

    xt_t = _ktile(np.ascontiguousarray(x.T), B).astype(NPBF16)
    # ctx^T chunk-major: [p, bc*(KT*NT) + k*NT + j] = context[bc*NT+j, k*128+p]
    ctx_t = np.ascontiguousarray(
        context.T.reshape(KT, 128, B // NT, NT).transpose(1, 2, 0, 3)
        .reshape(128, KT * B)).astype(NPBF16)
    cwt_t = _ktile(np.ascontiguousarray(ctx_w.T), H).astype(NPBF16)
    ctx_b_col = np.ascontiguousarray(ctx_b.reshape(H, 1))
    ln_g_bc = np.ascontiguousarray(np.broadcast_to(ln_g[None, :], (128, H)))
    ln_b_bc = np.ascontiguousarray(np.broadcast_to(ln_b[None, :], (128, H)))
    coeff_wT = np.ascontiguousarray(coeff_w.T)
    coeff_b_bc = np.ascontiguousarray(np.broadcast_to(coeff_b[None, :], (128, NB)))
    A_flatT = np.ascontiguousarray(basis_A.transpose(2, 0, 1).reshape(D, NB * RK))
    aT_t = _ktile(A_flatT, NB * RK).astype(NPBF16)

    C_PAD = N_CORES * CS
    bwT = np.zeros((D, C_PAD), np.float32)
    bwT[:, :C_FULL] = base_w.T
    Bm33 = np.zeros((NB * RK + 1, C_PAD), np.float32)
    Bm33[:NB * RK, :C_FULL] = basis_B.transpose(0, 2, 1).reshape(NB * RK, C_FULL)
    Bm33[NB * RK, :C_FULL] = base_b

    rep = {
        "xt_t": xt_t, "ctx_t": ctx_t, "cwt_t": cwt_t, "ctx_b_col": ctx_b_col,
        "ln_g_bc": ln_g_bc, "ln_b_bc": ln_b_bc, "coeff_wT": coeff_wT,
        "coeff_b_bc": coeff_b_bc, "aT_t": aT_t,
    }
    in_maps = []
    for c in range(N_CORES):
        sl = slice(c * CS, (c + 1) * CS)
        shard = bwT[:, sl]
        parts = []
        c0 = 0
        for W in C_TILES:
            parts.append(_ktile(np.ascontiguousarray(shard[:, c0:c0 + W]), W))
            c0 += W
        m = dict(rep)
        m["bwp"] = np.concatenate(parts, axis=1).astype(NPBF16)
        m["Bm33"] = np.ascontiguousarray(Bm33[:, sl]).astype(NPBF16)
        in_maps.append(m)
    return in_maps


def run(in_maps, **spmd_kwargs):
    nc = _get_program()
    res = run_bass_kernel_spmd(nc, in_maps, core_ids=list(range(N_CORES)),
                               **spmd_kwargs)
    out = np.concatenate([res.results[c]["out"] for c in range(N_CORES)], axis=1)
    return np.ascontiguousarray(out[:, :C_FULL]), res


def kernel(**inputs):
    in_maps = prepare_in_maps(**inputs)
    out, _ = run(in_maps)
    return out
